# revision 5
# baseline (speedup 1.0000x reference)
"""Trainium2 Bass kernel: memory-augmented attention block (12 heads, d=64).

Computation (per batch b):
    qkv = x @ w_attn + b_attn ; q,k,v split, 12 heads of 64
    a   = softmax(q k^T) v                      (no 1/sqrt(d) scaling)
    mkv = mem @ w_mem + b_mem ; mk,mv split
    a1  = softmax(q mk^T) mv
    alpha = sigmoid([a,a1] @ w_alpha + b_alpha)
    out = (alpha*a + (1-alpha)*a1) @ w_proj + b_proj

Sharding: data-parallel over (batch=2) x (512-row query blocks) = 8 cores, no
collectives.  Core c gets x[batch] ROTATED so its own 512 query rows are rows
0:512 (softmax is permutation-invariant over keys); each core recomputes K/V
for its whole batch locally.

v3: DMA/start restructure + SBUF gate accumulation.
  - All weights/activations are repacked on host into partition-major blobs
    so every DMA is a long contiguous per-partition read; descriptors are
    issued from four different engine queues (vector/sync/scalar/gpsimd) in
    demand order, with xq/wq/xr split per k-tile so the first matmuls start
    as soon as the first 0.33MB lands instead of after the full 2MB.
  - q-projection runs k-outer across 6 PSUM banks (starts on k-tile 0's
    arrival); kT chunk-0 production is interleaved with the memory-attention
    pairs to fill the exp-wait stalls.
  - The alpha gate is accumulated in SBUF f32 (DVE adds) as each head-pair's
    self-attention output is normalized, so the end-of-kernel tail is only
    pair 5's 6 gate matmuls + sigmoid/fuse/proj instead of all 72.

On-chip: feature-major ("transposed") activations [feat, seq].  Scores are
computed as P^T = [s_k, s_q]; softmax runs WITHOUT max subtraction (scores
~N(0,2.5), exp stays finite) and the denominator comes from a ones column
appended to V (M=65 trick).  Head pairs are packed into PE row groups
0:64/64:128 for the K=64 score matmuls (concurrent row-tiled execution).
The softmax denominator row is broadcast across the head's 64 partitions
with a K=1 ones-matmul on the PE, then reciprocal+multiply on DVE.  All
matmuls bf16 with f32 PSUM accumulation.
"""

import sys

if "/opt/trn_rl_repo" not in sys.path:
    sys.path.insert(0, "/opt/trn_rl_repo")

from contextlib import ExitStack

import numpy as np

import concourse.bass as bass
import concourse.bacc as bacc
import concourse.tile as tile
from concourse import mybir

F32 = mybir.dt.float32
BF16 = mybir.dt.bfloat16
AF = mybir.ActivationFunctionType
ALU = mybir.AluOpType

N_STATE = 768
N_HEAD = 12
DH = 64
M_SLOTS = 100
S = 2048          # keys per batch (= full batch sequence)
SQ = 512          # queries per core
P = 128
NF = N_STATE // P     # 6 feature tiles
NS = S // P           # 16 sequence chunks
NPAIR = N_HEAD // 2   # 6 head pairs
VW = DH + 1           # 65: v columns + ones column per head


def build_nc(debug: bool = False) -> bass.Bass:
    nc = bacc.Bacc(debug=debug)

    # All blobs are partition-major: [128, free] with long contiguous
    # per-partition rows, so each DMA descriptor moves KBs per partition.
    xq_ext = nc.declare_dram_parameter("xq", [P, NF * SQ], BF16, isOutput=False)
    xr_ext = nc.declare_dram_parameter("xr", [P, NF * (S - SQ)], BF16, isOutput=False)
    wq_ext = nc.declare_dram_parameter("wq", [P, NF * N_STATE], BF16, isOutput=False)
    wk_ext = nc.declare_dram_parameter("wk", [P, NF * NF * P], BF16, isOutput=False)
    wv_ext = nc.declare_dram_parameter("wv", [P, NF * N_STATE], BF16, isOutput=False)
    wa_ext = nc.declare_dram_parameter("wa", [P, 2 * NF * N_STATE], BF16, isOutput=False)
    wp_ext = nc.declare_dram_parameter("wp", [P, NF * N_STATE], BF16, isOutput=False)
    bcol_ext = nc.declare_dram_parameter("bcol", [P, 3 * NF], F32, isOutput=False)
    brow_ext = nc.declare_dram_parameter("brow", [2, N_STATE], F32, isOutput=False)
    mkT_ext = nc.declare_dram_parameter("mkT", [P, NF * P], BF16, isOutput=False)
    mv_ext = nc.declare_dram_parameter("mv_sb", [P, N_HEAD * VW], BF16, isOutput=False)
    out_ext = nc.declare_dram_parameter("out", [SQ, N_STATE], F32, isOutput=True)

    SR = S - SQ  # 1536 columns held in xr

    with ExitStack() as ctx:
        tc = ctx.enter_context(tile.TileContext(nc, pool_alloc_mode="queue"))

        const = ctx.enter_context(tc.tile_pool(name="const", bufs=1, side="left"))
        pearly = tc.alloc_tile_pool(name="pearly", bufs=1, side="left")
        wq_pool = tc.alloc_tile_pool(name="wq_pool", bufs=1, side="left")
        w_pool = tc.alloc_tile_pool(name="w_pool", bufs=1, side="left")
        w2_pool = tc.alloc_tile_pool(name="w2_pool", bufs=1, side="left")

        # ---- persistent activations -------------------------------------
        xq = pearly.tile([P, NF, SQ], BF16, name="xq")         # x^T cols 0:512
        xr = pearly.tile([P, NF, SR], BF16, name="xr")         # x^T cols 512:2048
        kT = pearly.tile([P, NF, S], BF16, name="kT")          # k^T  [feat, s]
        qT = pearly.tile([P, NF, SQ], BF16, name="qT")         # q^T  [feat, sq]
        v_sb = pearly.tile([P, NS, N_HEAD * VW], BF16, name="v_sb")   # v + ones col
        mkT = pearly.tile([P, NF, P], BF16, name="mkT")        # mk^T (cols >=100 zero)
        mv_sb = pearly.tile([P, N_HEAD * VW], BF16, name="mv_sb")
        wq_sb = wq_pool.tile([P, NF, N_STATE], BF16, name="wq_sb")
        wk_sb = w_pool.tile([P, NF, NF, P], BF16, name="wk_sb")   # [p, f, k, c]
        wv_sb = w_pool.tile([P, NF, N_STATE], BF16, name="wv_sb")
        wa_sb = w2_pool.tile([P, 2 * NF, N_STATE], BF16, name="wa_sb")
        wp_sb = w2_pool.tile([P, NF, N_STATE], BF16, name="wp_sb")
        bcol = const.tile([P, 3 * NF], F32, name="bcol")       # bq | bk | bal
        bv_row = const.tile([P, N_STATE], F32, name="bv_row")
        bp_row = const.tile([P, N_STATE], F32, name="bp_row")

        # ---- DMA issue: three engine queues in demand order ---------------
        # sync: xq+wq interleaved per k-tile so the k-outer q matmuls start
        # on the first 0.66MB instead of after the full 2MB
        for k in range(NF):
            nc.sync.dma_start(out=xq[:, k, :], in_=xq_ext[:, k * SQ:(k + 1) * SQ])
            nc.sync.dma_start(out=wq_sb[:, k, :],
                              in_=wq_ext[:, k * N_STATE:(k + 1) * N_STATE])
        # scalar: small constants for the memory branch + biases
        nc.scalar.dma_start(out=bcol, in_=bcol_ext[:, :])
        nc.scalar.dma_start(out=mkT, in_=mkT_ext.rearrange("p (f m) -> p f m", f=NF))
        nc.scalar.dma_start(out=mv_sb, in_=mv_ext[:, :])

        def row_bias(dst, row):
            src = brow_ext[row:row + 1, :]
            bcast = bass.AP(tensor=src.tensor, offset=src.offset,
                            ap=[[0, P]] + [list(src.ap[1])])
            nc.scalar.dma_start(out=dst, in_=bcast)

        row_bias(bv_row, 0)
        row_bias(bp_row, 1)
        # gpsimd: bulk stream in demand order
        nc.gpsimd.dma_start(out=wk_sb[:, 0], in_=wk_ext.rearrange(
            "p (f k c) -> p f k c", f=NF, k=NF)[:, 0])
        for k in range(NF):
            nc.gpsimd.dma_start(out=xr[:, k, :], in_=xr_ext[:, k * SR:(k + 1) * SR])
        nc.gpsimd.dma_start(out=wv_sb, in_=wv_ext.rearrange("p (k c) -> p k c", k=NF))
        nc.gpsimd.dma_start(out=wk_sb[:, 1:NF], in_=wk_ext.rearrange(
            "p (f k c) -> p f k c", f=NF, k=NF)[:, 1:NF])
        nc.gpsimd.dma_start(out=wa_sb[:, 0:NF], in_=wa_ext.rearrange(
            "p (j c) -> p j c", j=2 * NF)[:, 0:NF])
        nc.gpsimd.dma_start(out=wa_sb[:, NF:2 * NF], in_=wa_ext.rearrange(
            "p (j c) -> p j c", j=2 * NF)[:, NF:2 * NF])
        nc.gpsimd.dma_start(out=wp_sb, in_=wp_ext.rearrange("p (k c) -> p k c", k=NF))

        bq_col = bcol[:, 0:NF]
        bk_col = bcol[:, NF:2 * NF]
        bal_col = bcol[:, 2 * NF:3 * NF]
        bv3 = bv_row.rearrange("p (h w) -> p h w", h=N_HEAD)

        def xchunk_cols(k, lo, hi):
            # columns [lo:hi) of full-x k-tile, split across xq/xr
            if hi <= SQ:
                return xq[:, k, lo:hi]
            assert lo >= SQ
            return xr[:, k, lo - SQ:hi - SQ]

        # ---- q: k-outer over 6 PSUM banks (starts on k-tile 0 arrival) ---
        ps_q = tc.alloc_tile_pool(name="ps_q", bufs=NF, space="PSUM")
        qps = [ps_q.tile([P, SQ], F32, tag="q", name="qps%d" % f) for f in range(NF)]
        for k in range(NF):
            for f in range(NF):
                nc.tensor.matmul(qps[f], wq_sb[:, k, f * P:(f + 1) * P], xq[:, k, :],
                                 start=(k == 0), stop=(k == NF - 1))
        for f in range(NF):
            nc.vector.tensor_scalar_add(out=qT[:, f, :], in0=qps[f],
                                        scalar1=bq_col[:, f:f + 1])
        ps_q.release()

        ps_w = ctx.enter_context(tc.tile_pool(name="ps_w", bufs=2, space="PSUM"))

        def emit_kT(f):
            for n in range(4):
                ps = ps_w.tile([P, SQ], F32, tag="w")
                for k in range(NF):
                    nc.tensor.matmul(
                        ps, wk_sb[:, f, k, :], xchunk_cols(k, n * 512, (n + 1) * 512),
                        start=(k == 0), stop=(k == NF - 1))
                nc.vector.tensor_scalar_add(
                    out=kT[:, f, n * 512:(n + 1) * 512], in0=ps,
                    scalar1=bk_col[:, f:f + 1])

        def emit_v(m):
            v3 = v_sb[:, m, :].rearrange("p (h w) -> p h w", h=N_HEAD)
            for part in range(2):
                lo_f, n_h, h0p = (0, 8, 0) if part == 0 else (512, 4, 8)
                wid = n_h * DH
                ps = ps_w.tile([P, SQ], F32, tag="w")
                for k in range(NF):
                    nc.tensor.matmul(
                        ps[:, 0:wid], xchunk_cols(k, m * P, (m + 1) * P),
                        wv_sb[:, k, lo_f:lo_f + wid],
                        start=(k == 0), stop=(k == NF - 1))
                nc.vector.tensor_tensor(
                    out=v3[:, h0p:h0p + n_h, 0:DH],
                    in0=ps[:, 0:wid].rearrange("p (h w) -> p h w", h=n_h),
                    in1=bv3[:, h0p:h0p + n_h, :],
                    op=ALU.add)
            nc.vector.memset(v3[:, :, DH:VW], 1.0)

        # ==================================================================
        # Phase 2: attention (+ interleaved kT / v production)
        # ==================================================================
        plate = tc.alloc_tile_pool(name="plate", bufs=1, side="right")
        aT_bf = plate.tile([P, NF, SQ], BF16, name="aT_bf")
        a1T_bf = plate.tile([P, NF, SQ], BF16, name="a1T_bf")
        alphaT = plate.tile([P, NF, SQ], BF16, name="alphaT")
        dT_bf = plate.tile([P, NF, SQ], BF16, name="dT_bf")
        al_acc = plate.tile([P, NF, SQ], F32, name="al_acc")   # gate pre-activation
        ones_bf = plate.tile([VW, DH], BF16, name="ones_bf")
        nc.vector.memset(ones_bf, 1.0)

        ps_at = tc.alloc_tile_pool(name="ps_at", bufs=2, space="PSUM")
        expp = tc.alloc_tile_pool(name="expp", bufs=3, side="right")
        ps_kt = tc.alloc_tile_pool(name="ps_kt", bufs=4, space="PSUM")

        pslice = (slice(0, DH), slice(DH, P))

        def evict_norm_pair(at_ps, h0, h1, t, dst_bf):
            # Per head: stage psum -> bf16 SBUF (row 64 = softmax denominator),
            # broadcast the denominator row across the head's 64 partitions
            # with a K=1 ones-matmul, approx-reciprocal on DVE, then one
            # multiply.  h0 lands directly on partitions 0:64; h1 normalizes
            # in place and DMA-moves to partitions 64:128.  Both copies are
            # emitted before the matmuls so neither engine head-of-line
            # blocks the other.  No DRAM round-trips.
            evs, rps = [], []
            for hi, h in enumerate((h0, h1)):
                ev = expp.tile([VW, SQ], BF16, tag="ev", name="ev%d" % hi)
                if hi == 0:
                    nc.scalar.copy(out=ev, in_=at_ps[h])
                else:
                    nc.vector.tensor_copy(out=ev, in_=at_ps[h])
                evs.append(ev)
            for hi in range(2):
                rb_ps = ps_w.tile([P, SQ], F32, tag="w", name="rbps")
                nc.tensor.matmul(rb_ps[0:DH, :], ones_bf[DH:VW, 0:DH],
                                 evs[hi][DH:VW, :],
                                 start=True, stop=True, tile_position=(DH, 0))
                rps.append(rb_ps)
            for hi in range(2):
                rb = expp.tile([DH, SQ], F32, tag="rb", bufs=2, name="rb")
                nc.vector.reciprocal_approx_fast(out=rb, in_=rps[hi][0:DH, :])
                if hi == 0:
                    nc.vector.tensor_tensor(out=dst_bf[0:DH, t, :], in0=evs[0][0:DH, :],
                                            in1=rb, op=ALU.mult)
                else:
                    nc.vector.tensor_tensor(out=evs[1][0:DH, :], in0=evs[1][0:DH, :],
                                            in1=rb, op=ALU.mult)
                    nc.sync.dma_start(out=dst_bf[DH:P, t, :], in_=evs[1][0:DH, :])

        # ---- memory attention (needs only qT + tiny host-computed mkT/mv)
        # with kT chunk-0 production interleaved per pair: the 4 kT matmuls
        # of k-tile t run while the scalar engine computes pair t's exp, so
        # the PE doesn't stall on the exp->AV dependency.  Padded mem keys
        # 100:128 give exp(0)=1, killed by the zero rows of mv. ------------
        ktps = [ps_kt.tile([P, SQ], F32, tag="kt", name="ktps%d" % n)
                for n in range(4)]
        for t in range(NPAIR):
            h0, h1 = 2 * t, 2 * t + 1
            sc1 = {h0: ps_w.tile([P, SQ], F32, tag="w", name="msc0"),
                   h1: ps_w.tile([P, SQ], F32, tag="w", name="msc1")}
            for hi, h in enumerate((h0, h1)):
                nc.tensor.matmul(sc1[h], mkT[pslice[hi], t, :], qT[pslice[hi], t, :],
                                 start=True, stop=True)
            # kT f-tile 0, contraction k-tile t (k-outer across 4 banks)
            for n in range(4):
                nc.tensor.matmul(ktps[n], wk_sb[:, 0, t, :],
                                 xchunk_cols(t, n * 512, (n + 1) * 512),
                                 start=(t == 0), stop=(t == NPAIR - 1))
            a1_ps = {h0: ps_at.tile([VW, SQ], F32, tag="at_ps", name="a1t0"),
                     h1: ps_at.tile([VW, SQ], F32, tag="at_ps", name="a1t1")}
            for h in (h0, h1):
                ex1 = expp.tile([P, 1024], BF16, tag="ex", bufs=4, name="ex1m")
                nc.scalar.activation(out=ex1[:, 0:512], in_=sc1[h], func=AF.Exp)
                nc.tensor.matmul(a1_ps[h], mv_sb[:, h * VW:(h + 1) * VW],
                                 ex1[:, 0:512], start=True, stop=True)
            evict_norm_pair(a1_ps, h0, h1, t, a1T_bf)
        for n in range(4):
            nc.vector.tensor_scalar_add(
                out=kT[:, 0, n * 512:(n + 1) * 512], in0=ktps[n],
                scalar1=bk_col[:, 0:1])
        ps_kt.release()

        ps_sc = tc.alloc_tile_pool(name="ps_sc", bufs=2, space="PSUM")

        for t in range(NPAIR):
            h0, h1 = 2 * t, 2 * t + 1
            at_ps = {h0: ps_at.tile([VW, SQ], F32, tag="at_ps", name="at0"),
                     h1: ps_at.tile([VW, SQ], F32, tag="at_ps", name="at1")}
            for g in range(NS // 2):
                c0, c1 = 2 * g, 2 * g + 1
                if t == 0:
                    emit_v(c0)
                    emit_v(c1)
                sc = {h0: ps_sc.tile([P, 1024], F32, tag="sc", name="sc0"),
                      h1: ps_sc.tile([P, 1024], F32, tag="sc", name="sc1")}
                ex = {h0: expp.tile([P, 1024], BF16, tag="ex", bufs=4, name="ex0"),
                      h1: expp.tile([P, 1024], BF16, tag="ex", bufs=4, name="ex1")}
                for ci, c in enumerate((c0, c1)):
                    # head pair packed into PE row groups 0:64 / 64:128
                    for hi, h in enumerate((h0, h1)):
                        nc.tensor.matmul(sc[h][:, ci * 512:(ci + 1) * 512],
                                         kT[pslice[hi], t, c * P:(c + 1) * P],
                                         qT[pslice[hi], t, :],
                                         start=True, stop=True)
                for h in (h0, h1):
                    nc.scalar.activation(out=ex[h], in_=sc[h], func=AF.Exp)
                for ci, c in enumerate((c0, c1)):
                    for h in (h0, h1):
                        nc.tensor.matmul(
                            at_ps[h],
                            v_sb[:, c, h * VW:(h + 1) * VW],
                            ex[h][:, ci * 512:(ci + 1) * 512],
                            start=(c == 0), stop=(c == NS - 1))
            if t + 1 < NPAIR:
                emit_kT(t + 1)
            evict_norm_pair(at_ps, h0, h1, t, aT_bf)
            # d = a - a1, used by the final fuse (gate consumes original a/a1)
            nc.vector.tensor_tensor(out=dT_bf[:, t, :], in0=aT_bf[:, t, :],
                                    in1=a1T_bf[:, t, :], op=ALU.subtract)
            # gate contribution of this pair, accumulated in SBUF f32 so the
            # kernel tail only carries pair 5's 6 matmuls.  Pair 0's group
            # also folds in the whole a1 branch (available since mem phase).
            for f in range(NF):
                ps = ps_w.tile([P, SQ], F32, tag="w", name="alps")
                if t == 0:
                    nc.tensor.matmul(ps, wa_sb[:, 0, f * P:(f + 1) * P],
                                     aT_bf[:, 0, :], start=True, stop=False)
                    for j in range(NF):
                        nc.tensor.matmul(ps, wa_sb[:, NF + j, f * P:(f + 1) * P],
                                         a1T_bf[:, j, :],
                                         start=False, stop=(j == NF - 1))
                    nc.vector.tensor_copy(out=al_acc[:, f, :], in_=ps)
                else:
                    nc.tensor.matmul(ps, wa_sb[:, t, f * P:(f + 1) * P],
                                     aT_bf[:, t, :], start=True, stop=True)
                    nc.vector.tensor_tensor(out=al_acc[:, f, :], in0=al_acc[:, f, :],
                                            in1=ps, op=ALU.add)

        ps_sc.release()
        ps_at.release()

        # ==================================================================
        # Phase 3: sigmoid, fuse, project
        # ==================================================================
        for f in range(NF):
            nc.scalar.activation(out=alphaT[:, f, :], in_=al_acc[:, f, :],
                                 func=AF.Sigmoid, bias=bal_col[:, f:f + 1])
            # fused = a1 + alpha*d, per f-tile so it pipelines under the
            # next f's sigmoid
            nc.vector.tensor_tensor(out=dT_bf[:, f, :], in0=alphaT[:, f, :],
                                    in1=dT_bf[:, f, :], op=ALU.mult)
            nc.vector.tensor_tensor(out=a1T_bf[:, f, :], in0=a1T_bf[:, f, :],
                                    in1=dT_bf[:, f, :], op=ALU.add)
        fusedT = a1T_bf

        # out[m-block] = fused @ w_proj + b_proj   (natural layout, direct DMA)
        outp = tc.alloc_tile_pool(name="outp", bufs=2, side="right")
        for m in range(SQ // P):
            ot = outp.tile([P, N_STATE], F32, tag="ot")
            for part in range(2):
                lo_f = 0 if part == 0 else 512
                wid = 512 if part == 0 else 256
                ps = ps_w.tile([P, SQ], F32, tag="w")
                for k in range(NF):
                    nc.tensor.matmul(ps[:, 0:wid], fusedT[:, k, m * P:(m + 1) * P],
                                     wp_sb[:, k, lo_f:lo_f + wid],
                                     start=(k == 0), stop=(k == NF - 1))
                nc.vector.tensor_tensor(out=ot[:, lo_f:lo_f + wid], in0=ps[:, 0:wid],
                                        in1=bp_row[:, lo_f:lo_f + wid], op=ALU.add)
            nc.sync.dma_start(out=out_ext[m * P:(m + 1) * P, :], in_=ot)

        outp.release()
        expp.release()
        plate.release()
        w2_pool.release()
        w_pool.release()
        wq_pool.release()
        pearly.release()

    nc.compile()
    return nc


_NC = None


def _get_nc():
    global _NC
    if _NC is None:
        _NC = build_nc()
    return _NC


def _pm(a, ktiles):
    # [ktiles*128, C] -> partition-major [128, ktiles*C] blob
    c = a.shape[1]
    return np.ascontiguousarray(
        a.reshape(ktiles, P, c).transpose(1, 0, 2).reshape(P, ktiles * c))


def _build_in_maps(inputs):
    import ml_dtypes

    BF = ml_dtypes.bfloat16
    x = np.asarray(inputs["x"], dtype=np.float32)                 # [2,2048,768]
    mem = np.asarray(inputs["memory_features"], np.float32).reshape(M_SLOTS, N_STATE)
    w_mem = np.asarray(inputs["w_mem"], np.float32)
    b_mem = np.asarray(inputs["b_mem"], np.float32)
    w_attn = np.asarray(inputs["w_attn"], np.float32)
    b_attn = np.asarray(inputs["b_attn"], np.float32)

    # host-side memory-branch projections (tiny): mkv = mem @ w_mem + b_mem
    mkv = mem @ w_mem + b_mem
    mk, mv = mkv[:, :N_STATE], mkv[:, N_STATE:]
    mkT = np.zeros((N_STATE, P), np.float32)
    mkT[:, :M_SLOTS] = mk.T
    mv_sb = np.zeros((P, N_HEAD * VW), np.float32)
    for h in range(N_HEAD):
        mv_sb[:M_SLOTS, h * VW:h * VW + DH] = mv[:, h * DH:(h + 1) * DH]
        mv_sb[:M_SLOTS, h * VW + DH] = 1.0

    # weight blobs, partition-major.  wk additionally reordered f-major:
    # wk[p, f, k, c] = w_attn[k*128+p, 768 + f*128 + c]
    wq = _pm(w_attn[:, 0:N_STATE].astype(BF), NF)
    wkb = w_attn[:, N_STATE:2 * N_STATE].astype(BF).reshape(NF, P, NF, P)
    wk = np.ascontiguousarray(wkb.transpose(1, 2, 0, 3).reshape(P, NF * NF * P))
    wv = _pm(w_attn[:, 2 * N_STATE:3 * N_STATE].astype(BF), NF)
    wa = _pm(np.asarray(inputs["w_alpha"], np.float32).astype(BF), 2 * NF)
    wp = _pm(np.asarray(inputs["w_proj"], np.float32).astype(BF), NF)

    bcol = np.empty((P, 3 * NF), np.float32)
    bcol[:, 0:NF] = b_attn[0:N_STATE].reshape(NF, P).T
    bcol[:, NF:2 * NF] = b_attn[N_STATE:2 * N_STATE].reshape(NF, P).T
    bcol[:, 2 * NF:3 * NF] = np.asarray(inputs["b_alpha"], np.float32).reshape(NF, P).T
    brow = np.stack([b_attn[2 * N_STATE:3 * N_STATE],
                     np.asarray(inputs["b_proj"], np.float32)])

    common = {
        "wq": wq, "wk": wk, "wv": wv, "wa": wa, "wp": wp,
        "bcol": np.ascontiguousarray(bcol),
        "brow": np.ascontiguousarray(brow),
        "mkT": _pm(mkT.astype(BF), NF),
        "mv_sb": np.ascontiguousarray(mv_sb.astype(BF)),
    }

    in_maps = []
    for c in range(8):
        b, j = c // 4, c % 4
        xb = np.roll(x[b], -SQ * j, axis=0).T.astype(BF)          # [768, 2048]
        in_maps.append({
            "xq": _pm(xb[:, 0:SQ], NF),
            "xr": _pm(xb[:, SQ:S], NF),
            **common,
        })
    return in_maps


def kernel(**inputs) -> np.ndarray:
    from concourse.bass_utils import run_bass_kernel_spmd

    nc = _get_nc()
    in_maps = _build_in_maps(inputs)
    res = run_bass_kernel_spmd(nc, in_maps, core_ids=list(range(8))).results
    B = np.asarray(inputs["x"]).shape[0]
    out = np.empty((B, S, N_STATE), dtype=np.float32)
    for c in range(8):
        b, j = c // 4, c % 4
        out[b, SQ * j:SQ * (j + 1)] = res[c]["out"]
    return out


# revision 8
# speedup vs baseline: 1.0702x; 1.0702x over previous
"""Trainium2 Bass kernel: memory-augmented attention block (12 heads, d=64).

Computation (per batch b):
    qkv = x @ w_attn + b_attn ; q,k,v split, 12 heads of 64
    a   = softmax(q k^T) v                      (no 1/sqrt(d) scaling)
    mkv = mem @ w_mem + b_mem ; mk,mv split
    a1  = softmax(q mk^T) mv
    alpha = sigmoid([a,a1] @ w_alpha + b_alpha)
    out = (alpha*a + (1-alpha)*a1) @ w_proj + b_proj

Sharding: data-parallel over (batch=2) x (512-row query blocks) = 8 cores, no
collectives.  Core c gets x[batch] ROTATED so its own 512 query rows are rows
0:512 (softmax is permutation-invariant over keys); each core recomputes K/V
for its whole batch locally.

v3: DMA/start restructure + SBUF gate accumulation.
  - All weights/activations are repacked on host into partition-major blobs
    so every DMA is a long contiguous per-partition read; descriptors are
    issued from four different engine queues (vector/sync/scalar/gpsimd) in
    demand order, with xq/wq/xr split per k-tile so the first matmuls start
    as soon as the first 0.33MB lands instead of after the full 2MB.
  - q-projection runs k-outer across 6 PSUM banks (starts on k-tile 0's
    arrival); kT chunk-0 production is interleaved with the memory-attention
    pairs to fill the exp-wait stalls.
  - The alpha gate is accumulated in SBUF f32 (DVE adds) as each head-pair's
    self-attention output is normalized, so the end-of-kernel tail is only
    pair 5's 6 gate matmuls + sigmoid/fuse/proj instead of all 72.

On-chip: feature-major ("transposed") activations [feat, seq].  Scores are
computed as P^T = [s_k, s_q]; softmax runs WITHOUT max subtraction (scores
~N(0,2.5), exp stays finite) and the denominator comes from a ones column
appended to V (M=65 trick).  Head pairs are packed into PE row groups
0:64/64:128 for the K=64 score matmuls (concurrent row-tiled execution).
The softmax denominator row is broadcast across the head's 64 partitions
with a K=1 ones-matmul on the PE, then reciprocal+multiply on DVE.  All
matmuls bf16 with f32 PSUM accumulation.
"""

import sys

if "/opt/trn_rl_repo" not in sys.path:
    sys.path.insert(0, "/opt/trn_rl_repo")

from contextlib import ExitStack

import numpy as np

import concourse.bass as bass
import concourse.bacc as bacc
import concourse.tile as tile
from concourse import mybir

F32 = mybir.dt.float32
BF16 = mybir.dt.bfloat16
AF = mybir.ActivationFunctionType
ALU = mybir.AluOpType

N_STATE = 768
N_HEAD = 12
DH = 64
M_SLOTS = 100
S = 2048          # keys per batch (= full batch sequence)
SQ = 512          # queries per core
P = 128
NF = N_STATE // P     # 6 feature tiles
NS = S // P           # 16 sequence chunks
NPAIR = N_HEAD // 2   # 6 head pairs
VW = DH + 1           # 65: v columns + ones column per head


def build_nc(debug: bool = False) -> bass.Bass:
    nc = bacc.Bacc(debug=debug)

    # All blobs are partition-major: [128, free] with long contiguous
    # per-partition rows, so each DMA descriptor moves KBs per partition.
    # qe: per k-tile, x columns 0:512 and w_q rows interleaved: [x_k | wq_k]
    qe_ext = nc.declare_dram_parameter("qe", [P, NF * (SQ + N_STATE)], BF16,
                                       isOutput=False)
    xr_ext = nc.declare_dram_parameter("xr", [P, NF * (S - SQ)], BF16, isOutput=False)
    wk_ext = nc.declare_dram_parameter("wk", [P, NF * NF * P], BF16, isOutput=False)
    wv_ext = nc.declare_dram_parameter("wv", [P, NF * N_STATE], BF16, isOutput=False)
    wa_ext = nc.declare_dram_parameter("wa", [P, 2 * NF * N_STATE], BF16, isOutput=False)
    wp_ext = nc.declare_dram_parameter("wp", [P, NF * N_STATE], BF16, isOutput=False)
    bcol_ext = nc.declare_dram_parameter("bcol", [P, 3 * NF], F32, isOutput=False)
    brow_ext = nc.declare_dram_parameter("brow", [2, N_STATE], F32, isOutput=False)
    mkT_ext = nc.declare_dram_parameter("mkT", [P, NF * P], BF16, isOutput=False)
    mv_ext = nc.declare_dram_parameter("mv_sb", [P, N_HEAD * VW], BF16, isOutput=False)
    out_ext = nc.declare_dram_parameter("out", [SQ, N_STATE], F32, isOutput=True)

    SR = S - SQ  # 1536 columns held in xr
    QE = SQ + N_STATE  # 1280: per-k [x_k | wq_k] row

    with ExitStack() as ctx:
        tc = ctx.enter_context(tile.TileContext(nc, pool_alloc_mode="queue"))

        const = ctx.enter_context(tc.tile_pool(name="const", bufs=1, side="left"))
        pearly = tc.alloc_tile_pool(name="pearly", bufs=1, side="left")
        w_pool = tc.alloc_tile_pool(name="w_pool", bufs=1, side="left")
        w2_pool = tc.alloc_tile_pool(name="w2_pool", bufs=1, side="left")

        # ---- persistent activations -------------------------------------
        qe = pearly.tile([P, NF, SQ + N_STATE], BF16, name="qe")  # [x_k | wq_k]
        xr = pearly.tile([P, NF, SR], BF16, name="xr")         # x^T cols 512:2048
        kT = pearly.tile([P, NF, S], BF16, name="kT")          # k^T  [feat, s]
        qT = pearly.tile([P, NF, SQ], BF16, name="qT")         # q^T  [feat, sq]
        v_sb = pearly.tile([P, NS, N_HEAD * VW], BF16, name="v_sb")   # v + ones col
        mkT = pearly.tile([P, NF, P], BF16, name="mkT")        # mk^T (cols >=100 zero)
        mv_sb = pearly.tile([P, N_HEAD * VW], BF16, name="mv_sb")
        wk_sb = w_pool.tile([P, NF, NF, P], BF16, name="wk_sb")   # [p, f, k, c]
        wv_sb = w_pool.tile([P, NF, N_STATE], BF16, name="wv_sb")
        wa_sb = w2_pool.tile([P, 2 * NF, N_STATE], BF16, name="wa_sb")
        wp_sb = w2_pool.tile([P, NF, N_STATE], BF16, name="wp_sb")
        bcol = const.tile([P, 3 * NF], F32, name="bcol")       # bq | bk | bal
        bv_row = const.tile([P, N_STATE], F32, name="bv_row")
        bp_row = const.tile([P, N_STATE], F32, name="bp_row")

        # ---- DMA issue: gpsimd carries everything bulk (its SWDGE queue
        # drains at ~300 GB/s; the sync-engine queue only manages ~15-60 GB/s
        # so it gets nothing bandwidth-critical).  Order = demand order, with
        # the q-phase inputs split per k-tile so matmuls start on the first
        # 0.33MB.  scalar queue (~85 GB/s) takes the small constants. -------
        for k in range(NF):
            nc.gpsimd.dma_start(out=qe[:, k, :],
                                in_=qe_ext[:, k * QE:(k + 1) * QE])
        nc.gpsimd.dma_start(out=wk_sb[:, 0], in_=wk_ext.rearrange(
            "p (f k c) -> p f k c", f=NF, k=NF)[:, 0])
        for k in range(NF):
            nc.gpsimd.dma_start(out=xr[:, k, :], in_=xr_ext[:, k * SR:(k + 1) * SR])
        nc.gpsimd.dma_start(out=wv_sb, in_=wv_ext.rearrange("p (k c) -> p k c", k=NF))
        nc.gpsimd.dma_start(out=wk_sb[:, 1:NF], in_=wk_ext.rearrange(
            "p (f k c) -> p f k c", f=NF, k=NF)[:, 1:NF])
        nc.gpsimd.dma_start(out=wa_sb[:, 0:NF], in_=wa_ext.rearrange(
            "p (j c) -> p j c", j=2 * NF)[:, 0:NF])
        nc.gpsimd.dma_start(out=wa_sb[:, NF:2 * NF], in_=wa_ext.rearrange(
            "p (j c) -> p j c", j=2 * NF)[:, NF:2 * NF])
        nc.gpsimd.dma_start(out=wp_sb, in_=wp_ext.rearrange("p (k c) -> p k c", k=NF))
        # scalar: small constants for the memory branch + biases
        nc.scalar.dma_start(out=bcol, in_=bcol_ext[:, :])
        nc.scalar.dma_start(out=mkT, in_=mkT_ext.rearrange("p (f m) -> p f m", f=NF))
        nc.scalar.dma_start(out=mv_sb, in_=mv_ext[:, :])

        def row_bias(dst, row):
            src = brow_ext[row:row + 1, :]
            bcast = bass.AP(tensor=src.tensor, offset=src.offset,
                            ap=[[0, P]] + [list(src.ap[1])])
            nc.scalar.dma_start(out=dst, in_=bcast)

        row_bias(bv_row, 0)
        row_bias(bp_row, 1)

        bq_col = bcol[:, 0:NF]
        bk_col = bcol[:, NF:2 * NF]
        bal_col = bcol[:, 2 * NF:3 * NF]
        bv3 = bv_row.rearrange("p (h w) -> p h w", h=N_HEAD)

        def xchunk_cols(k, lo, hi):
            # columns [lo:hi) of full-x k-tile, split across qe/xr
            if hi <= SQ:
                return qe[:, k, lo:hi]
            assert lo >= SQ
            return xr[:, k, lo - SQ:hi - SQ]

        # ---- q: k-outer over 6 PSUM banks (starts on k-tile 0 arrival) ---
        ps_q = tc.alloc_tile_pool(name="ps_q", bufs=NF, space="PSUM")
        qps = [ps_q.tile([P, SQ], F32, tag="q", name="qps%d" % f) for f in range(NF)]
        for k in range(NF):
            for f in range(NF):
                nc.tensor.matmul(qps[f], qe[:, k, SQ + f * P:SQ + (f + 1) * P],
                                 qe[:, k, 0:SQ],
                                 start=(k == 0), stop=(k == NF - 1))
        for f in range(NF):
            nc.vector.tensor_scalar_add(out=qT[:, f, :], in0=qps[f],
                                        scalar1=bq_col[:, f:f + 1])
        ps_q.release()

        ps_w = ctx.enter_context(tc.tile_pool(name="ps_w", bufs=2, space="PSUM"))

        def emit_kT(f):
            for n in range(4):
                ps = ps_w.tile([P, SQ], F32, tag="w")
                for k in range(NF):
                    nc.tensor.matmul(
                        ps, wk_sb[:, f, k, :], xchunk_cols(k, n * 512, (n + 1) * 512),
                        start=(k == 0), stop=(k == NF - 1))
                nc.vector.tensor_scalar_add(
                    out=kT[:, f, n * 512:(n + 1) * 512], in0=ps,
                    scalar1=bk_col[:, f:f + 1])

        def emit_v(m):
            v3 = v_sb[:, m, :].rearrange("p (h w) -> p h w", h=N_HEAD)
            for part in range(2):
                lo_f, n_h, h0p = (0, 8, 0) if part == 0 else (512, 4, 8)
                wid = n_h * DH
                ps = ps_w.tile([P, SQ], F32, tag="w")
                for k in range(NF):
                    nc.tensor.matmul(
                        ps[:, 0:wid], xchunk_cols(k, m * P, (m + 1) * P),
                        wv_sb[:, k, lo_f:lo_f + wid],
                        start=(k == 0), stop=(k == NF - 1))
                nc.vector.tensor_tensor(
                    out=v3[:, h0p:h0p + n_h, 0:DH],
                    in0=ps[:, 0:wid].rearrange("p (h w) -> p h w", h=n_h),
                    in1=bv3[:, h0p:h0p + n_h, :],
                    op=ALU.add)
            nc.vector.memset(v3[:, :, DH:VW], 1.0)

        # ==================================================================
        # Phase 2: attention (+ interleaved kT / v production)
        # ==================================================================
        plate = tc.alloc_tile_pool(name="plate", bufs=1, side="right")
        aT_bf = plate.tile([P, NF, SQ], BF16, name="aT_bf")
        a1T_bf = plate.tile([P, NF, SQ], BF16, name="a1T_bf")
        alphaT = plate.tile([P, NF, SQ], BF16, name="alphaT")
        dT_bf = plate.tile([P, NF, SQ], BF16, name="dT_bf")
        al_acc = plate.tile([P, NF, SQ], F32, name="al_acc")   # gate pre-activation
        ones_bf = plate.tile([VW, DH], BF16, name="ones_bf")
        nc.vector.memset(ones_bf, 1.0)

        ps_at = tc.alloc_tile_pool(name="ps_at", bufs=2, space="PSUM")
        expp = tc.alloc_tile_pool(name="expp", bufs=3, side="right")
        ps_kt = tc.alloc_tile_pool(name="ps_kt", bufs=4, space="PSUM")

        pslice = (slice(0, DH), slice(DH, P))

        def evict_norm_pair(at_ps, h0, h1, t, dst_bf):
            # Per head: stage psum -> bf16 SBUF (row 64 = softmax denominator),
            # broadcast the denominator row across the head's 64 partitions
            # with a K=1 ones-matmul, approx-reciprocal on DVE, then one
            # multiply.  h0 lands directly on partitions 0:64; h1 normalizes
            # in place and DMA-moves to partitions 64:128.  Both copies are
            # emitted before the matmuls so neither engine head-of-line
            # blocks the other.  No DRAM round-trips.
            evs, rps = [], []
            for hi, h in enumerate((h0, h1)):
                ev = expp.tile([VW, SQ], BF16, tag="ev", name="ev%d" % hi)
                if hi == 0:
                    nc.scalar.copy(out=ev, in_=at_ps[h])
                else:
                    nc.vector.tensor_copy(out=ev, in_=at_ps[h])
                evs.append(ev)
            for hi in range(2):
                rb_ps = ps_w.tile([P, SQ], F32, tag="w", name="rbps")
                nc.tensor.matmul(rb_ps[0:DH, :], ones_bf[DH:VW, 0:DH],
                                 evs[hi][DH:VW, :],
                                 start=True, stop=True, tile_position=(DH, 0))
                rps.append(rb_ps)
            for hi in range(2):
                rb = expp.tile([DH, SQ], F32, tag="rb", bufs=2, name="rb")
                nc.vector.reciprocal_approx_fast(out=rb, in_=rps[hi][0:DH, :])
                if hi == 0:
                    nc.vector.tensor_tensor(out=dst_bf[0:DH, t, :], in0=evs[0][0:DH, :],
                                            in1=rb, op=ALU.mult)
                else:
                    nc.vector.tensor_tensor(out=evs[1][0:DH, :], in0=evs[1][0:DH, :],
                                            in1=rb, op=ALU.mult)
                    nc.gpsimd.dma_start(out=dst_bf[DH:P, t, :], in_=evs[1][0:DH, :])

        # ---- memory attention (needs only qT + tiny host-computed mkT/mv)
        # with kT chunk-0 production interleaved per pair: the 4 kT matmuls
        # of k-tile t run while the scalar engine computes pair t's exp, so
        # the PE doesn't stall on the exp->AV dependency.  Padded mem keys
        # 100:128 give exp(0)=1, killed by the zero rows of mv. ------------
        ktps = [ps_kt.tile([P, SQ], F32, tag="kt", name="ktps%d" % n)
                for n in range(4)]
        for t in range(NPAIR):
            h0, h1 = 2 * t, 2 * t + 1
            sc1 = {h0: ps_w.tile([P, SQ], F32, tag="w", name="msc0"),
                   h1: ps_w.tile([P, SQ], F32, tag="w", name="msc1")}
            for hi, h in enumerate((h0, h1)):
                nc.tensor.matmul(sc1[h], mkT[pslice[hi], t, :], qT[pslice[hi], t, :],
                                 start=True, stop=True)
            # kT f-tile 0, contraction k-tile t (k-outer across 4 banks)
            for n in range(4):
                nc.tensor.matmul(ktps[n], wk_sb[:, 0, t, :],
                                 xchunk_cols(t, n * 512, (n + 1) * 512),
                                 start=(t == 0), stop=(t == NPAIR - 1))
            a1_ps = {h0: ps_at.tile([VW, SQ], F32, tag="at_ps", name="a1t0"),
                     h1: ps_at.tile([VW, SQ], F32, tag="at_ps", name="a1t1")}
            for h in (h0, h1):
                ex1 = expp.tile([P, 1024], BF16, tag="ex", bufs=4, name="ex1m")
                nc.scalar.activation(out=ex1[:, 0:512], in_=sc1[h], func=AF.Exp)
                nc.tensor.matmul(a1_ps[h], mv_sb[:, h * VW:(h + 1) * VW],
                                 ex1[:, 0:512], start=True, stop=True)
            evict_norm_pair(a1_ps, h0, h1, t, a1T_bf)
        for n in range(4):
            nc.vector.tensor_scalar_add(
                out=kT[:, 0, n * 512:(n + 1) * 512], in0=ktps[n],
                scalar1=bk_col[:, 0:1])
        ps_kt.release()

        ps_sc = tc.alloc_tile_pool(name="ps_sc", bufs=2, space="PSUM")

        for t in range(NPAIR):
            h0, h1 = 2 * t, 2 * t + 1
            at_ps = {h0: ps_at.tile([VW, SQ], F32, tag="at_ps", name="at0"),
                     h1: ps_at.tile([VW, SQ], F32, tag="at_ps", name="at1")}
            for g in range(NS // 2):
                c0, c1 = 2 * g, 2 * g + 1
                if t == 0:
                    emit_v(c0)
                    emit_v(c1)
                sc = {h0: ps_sc.tile([P, 1024], F32, tag="sc", name="sc0"),
                      h1: ps_sc.tile([P, 1024], F32, tag="sc", name="sc1")}
                ex = {h0: expp.tile([P, 1024], BF16, tag="ex", bufs=4, name="ex0"),
                      h1: expp.tile([P, 1024], BF16, tag="ex", bufs=4, name="ex1")}
                for ci, c in enumerate((c0, c1)):
                    # head pair packed into PE row groups 0:64 / 64:128
                    for hi, h in enumerate((h0, h1)):
                        nc.tensor.matmul(sc[h][:, ci * 512:(ci + 1) * 512],
                                         kT[pslice[hi], t, c * P:(c + 1) * P],
                                         qT[pslice[hi], t, :],
                                         start=True, stop=True)
                for h in (h0, h1):
                    nc.scalar.activation(out=ex[h], in_=sc[h], func=AF.Exp)
                for ci, c in enumerate((c0, c1)):
                    for h in (h0, h1):
                        nc.tensor.matmul(
                            at_ps[h],
                            v_sb[:, c, h * VW:(h + 1) * VW],
                            ex[h][:, ci * 512:(ci + 1) * 512],
                            start=(c == 0), stop=(c == NS - 1))
            if t + 1 < NPAIR:
                emit_kT(t + 1)
            evict_norm_pair(at_ps, h0, h1, t, aT_bf)
            # d = a - a1, used by the final fuse (gate consumes original a/a1)
            nc.vector.tensor_tensor(out=dT_bf[:, t, :], in0=aT_bf[:, t, :],
                                    in1=a1T_bf[:, t, :], op=ALU.subtract)
            # gate contribution of this pair, accumulated in SBUF f32 so the
            # kernel tail only carries pair 5's 6 matmuls.  Pair 0's group
            # also folds in the whole a1 branch (available since mem phase).
            for f in range(NF):
                ps = ps_w.tile([P, SQ], F32, tag="w", name="alps")
                if t == 0:
                    nc.tensor.matmul(ps, wa_sb[:, 0, f * P:(f + 1) * P],
                                     aT_bf[:, 0, :], start=True, stop=False)
                    for j in range(NF):
                        nc.tensor.matmul(ps, wa_sb[:, NF + j, f * P:(f + 1) * P],
                                         a1T_bf[:, j, :],
                                         start=False, stop=(j == NF - 1))
                    nc.vector.tensor_copy(out=al_acc[:, f, :], in_=ps)
                else:
                    nc.tensor.matmul(ps, wa_sb[:, t, f * P:(f + 1) * P],
                                     aT_bf[:, t, :], start=True, stop=True)
                    nc.vector.tensor_tensor(out=al_acc[:, f, :], in0=al_acc[:, f, :],
                                            in1=ps, op=ALU.add)

        ps_sc.release()
        ps_at.release()

        # ==================================================================
        # Phase 3: sigmoid, fuse, project
        # ==================================================================
        for f in range(NF):
            nc.scalar.activation(out=alphaT[:, f, :], in_=al_acc[:, f, :],
                                 func=AF.Sigmoid, bias=bal_col[:, f:f + 1])
            # fused = a1 + alpha*d, per f-tile so it pipelines under the
            # next f's sigmoid
            nc.vector.tensor_tensor(out=dT_bf[:, f, :], in0=alphaT[:, f, :],
                                    in1=dT_bf[:, f, :], op=ALU.mult)
            nc.vector.tensor_tensor(out=a1T_bf[:, f, :], in0=a1T_bf[:, f, :],
                                    in1=dT_bf[:, f, :], op=ALU.add)
        fusedT = a1T_bf

        # out[m-block] = fused @ w_proj + b_proj   (natural layout, direct DMA)
        outp = tc.alloc_tile_pool(name="outp", bufs=2, side="right")
        for m in range(SQ // P):
            ot = outp.tile([P, N_STATE], F32, tag="ot")
            for part in range(2):
                lo_f = 0 if part == 0 else 512
                wid = 512 if part == 0 else 256
                ps = ps_w.tile([P, SQ], F32, tag="w")
                for k in range(NF):
                    nc.tensor.matmul(ps[:, 0:wid], fusedT[:, k, m * P:(m + 1) * P],
                                     wp_sb[:, k, lo_f:lo_f + wid],
                                     start=(k == 0), stop=(k == NF - 1))
                nc.vector.tensor_tensor(out=ot[:, lo_f:lo_f + wid], in0=ps[:, 0:wid],
                                        in1=bp_row[:, lo_f:lo_f + wid], op=ALU.add)
            nc.gpsimd.dma_start(out=out_ext[m * P:(m + 1) * P, :], in_=ot)

        outp.release()
        expp.release()
        plate.release()
        w2_pool.release()
        w_pool.release()
        pearly.release()

    nc.compile()
    return nc


_NC = None


def _get_nc():
    global _NC
    if _NC is None:
        _NC = build_nc()
    return _NC


def _pm(a, ktiles):
    # [ktiles*128, C] -> partition-major [128, ktiles*C] blob
    c = a.shape[1]
    return np.ascontiguousarray(
        a.reshape(ktiles, P, c).transpose(1, 0, 2).reshape(P, ktiles * c))


def _build_in_maps(inputs):
    import ml_dtypes

    BF = ml_dtypes.bfloat16
    x = np.asarray(inputs["x"], dtype=np.float32)                 # [2,2048,768]
    mem = np.asarray(inputs["memory_features"], np.float32).reshape(M_SLOTS, N_STATE)
    w_mem = np.asarray(inputs["w_mem"], np.float32)
    b_mem = np.asarray(inputs["b_mem"], np.float32)
    w_attn = np.asarray(inputs["w_attn"], np.float32)
    b_attn = np.asarray(inputs["b_attn"], np.float32)

    # host-side memory-branch projections (tiny): mkv = mem @ w_mem + b_mem
    mkv = mem @ w_mem + b_mem
    mk, mv = mkv[:, :N_STATE], mkv[:, N_STATE:]
    mkT = np.zeros((N_STATE, P), np.float32)
    mkT[:, :M_SLOTS] = mk.T
    mv_sb = np.zeros((P, N_HEAD * VW), np.float32)
    for h in range(N_HEAD):
        mv_sb[:M_SLOTS, h * VW:h * VW + DH] = mv[:, h * DH:(h + 1) * DH]
        mv_sb[:M_SLOTS, h * VW + DH] = 1.0

    # weight blobs, partition-major.  wk additionally reordered f-major:
    # wk[p, f, k, c] = w_attn[k*128+p, 768 + f*128 + c]
    wq = _pm(w_attn[:, 0:N_STATE].astype(BF), NF).reshape(P, NF, N_STATE)
    wkb = w_attn[:, N_STATE:2 * N_STATE].astype(BF).reshape(NF, P, NF, P)
    wk = np.ascontiguousarray(wkb.transpose(1, 2, 0, 3).reshape(P, NF * NF * P))
    wv = _pm(w_attn[:, 2 * N_STATE:3 * N_STATE].astype(BF), NF)
    wa = _pm(np.asarray(inputs["w_alpha"], np.float32).astype(BF), 2 * NF)
    wp = _pm(np.asarray(inputs["w_proj"], np.float32).astype(BF), NF)

    bcol = np.empty((P, 3 * NF), np.float32)
    bcol[:, 0:NF] = b_attn[0:N_STATE].reshape(NF, P).T
    bcol[:, NF:2 * NF] = b_attn[N_STATE:2 * N_STATE].reshape(NF, P).T
    bcol[:, 2 * NF:3 * NF] = np.asarray(inputs["b_alpha"], np.float32).reshape(NF, P).T
    brow = np.stack([b_attn[2 * N_STATE:3 * N_STATE],
                     np.asarray(inputs["b_proj"], np.float32)])

    common = {
        "wk": wk, "wv": wv, "wa": wa, "wp": wp,
        "bcol": np.ascontiguousarray(bcol),
        "brow": np.ascontiguousarray(brow),
        "mkT": _pm(mkT.astype(BF), NF),
        "mv_sb": np.ascontiguousarray(mv_sb.astype(BF)),
    }

    in_maps = []
    for c in range(8):
        b, j = c // 4, c % 4
        xb = np.roll(x[b], -SQ * j, axis=0).T.astype(BF)          # [768, 2048]
        xqb = _pm(xb[:, 0:SQ], NF).reshape(P, NF, SQ)
        qe = np.concatenate([xqb, wq], axis=2).reshape(P, NF * (SQ + N_STATE))
        in_maps.append({
            "qe": np.ascontiguousarray(qe),
            "xr": _pm(xb[:, SQ:S], NF),
            **common,
        })
    return in_maps


def kernel(**inputs) -> np.ndarray:
    from concourse.bass_utils import run_bass_kernel_spmd

    nc = _get_nc()
    in_maps = _build_in_maps(inputs)
    res = run_bass_kernel_spmd(nc, in_maps, core_ids=list(range(8))).results
    B = np.asarray(inputs["x"]).shape[0]
    out = np.empty((B, S, N_STATE), dtype=np.float32)
    for c in range(8):
        b, j = c // 4, c % 4
        out[b, SQ * j:SQ * (j + 1)] = res[c]["out"]
    return out
